# revision 15
# baseline (speedup 1.0000x reference)
"""Trainium2 Bass kernel for the ContractiveREN problem.

Strategy
--------
Data parallel over the batch: each of the 8 NeuronCores gets a 2048-row
shard of ``u_in``; all (small) parameter matrices are folded on the host
into four 128x128 fp16 matmul weights plus two per-partition fp32 bias
vectors.

Math
----
The reference computes (per batch row u, with x0 the initial state):
    w_i   = tanh((xc_i + ud_i + sum_{j<i} D11_ij w_j) / Lam_i)   (i = 0..127)
    y     = u @ Gu^T + w @ Gw^T + c0
where everything except the w-recurrence is affine in (u, w) and folds into
    Lhat = D11 / Lam[:,None],  xcl = xc/Lam,  UD = (D12/Lam) @ u^T
    Gu   = C2 @ inv(E) @ B2 + D22,  Gw = C2 @ inv(E) @ B1 + D21
    c0   = C2 @ inv(E) @ F @ x0
The strictly-lower-triangular recurrence is solved by fixed-point
iteration  W <- tanh(Lhat @ W + UD + xcl), which contracts the error by
~3.2x per sweep; 3 sweeps give rel err ~3.4e-3 against the fp32
reference (the harness gate is 2e-2; measured hw error matches the host
fp16 simulation exactly).

Implementation notes (all fp16 data / fp32 PSUM accumulation):
  * Input loads avoid both row-granular DMA (descriptor-rate bound) and
    the XBAR DMA transpose (slow, and serializing against every other
    DMA): the host pre-scatters u and the constants so a handful of
    big-descriptor DMAs land them in SBUF as 128x128 blocks, which PE
    block-transposes (identity generated on device via affine_select)
    and cheap contiguous DVE copies reassemble feature-major. Each
    512-batch chunk rides its own DMA so the pipeline starts on the
    first chunk while the rest are in flight.
  * Each 512-batch chunk owns one live PSUM bank holding
    UD + Lhat@W_k: pass k accumulates Lhat@(W_k - W_{k-1}) via matmul
    (the W-delta is a cheap all-fp16 DVE subtract), and every tanh
    applies xcl as the ACT bias, so no UDb tensor and no fp32 adds.
  * Output is computed feature-major at each chunk's tail (Gu@Ut +
    Gw@W accumulated into the chunk's freed ps bank, + c0 via DVE
    tensor-scalar / ACT-Identity-bias alternating) and stored to DRAM
    feature-major as fp16; the host transposes and upcasts per core.
"""

import numpy as np

import concourse.bass as bass
import concourse.mybir as mybir
import concourse.tile as tile
from concourse import bacc
from concourse.bass_utils import run_bass_kernel_spmd

B = 16384
N_CORES = 8
BC = B // N_CORES  # 2048 batch rows per core
DIM_IN = 128
DIM_OUT = 128
DIM_X = 512
DIM_NL = 128
DIM_H = 2 * DIM_X + DIM_NL
EPS = 1e-3
ALPHA = 1.0
NCH = 4            # 512-column batch chunks (one PSUM bank each)
CW = BC // NCH     # 512
M_PASSES = 2       # delta-Jacobi passes after the seeded first sweep
CROWS = 5 * 128    # constant region: D12L | Lhat | Gu | Gw | bias blocks
F16 = mybir.dt.float16
F32 = mybir.dt.float32
TANH = mybir.ActivationFunctionType.Tanh
IDENT = mybir.ActivationFunctionType.Identity
EQ = mybir.AluOpType.is_equal

_BUILT = {}


def _build_nc():
    nc = bacc.Bacc("TRN2", target_bir_lowering=False, debug=False)
    ub = nc.dram_tensor(
        "ub", [CROWS + BC, DIM_IN], F16, kind="ExternalInput"
    ).ap()
    y = nc.dram_tensor("y", [DIM_OUT, BC], F16, kind="ExternalOutput").ap()

    # big-descriptor views: partition p reads R consecutive DRAM rows,
    # giving SBUF col (r*128 + f) = DRAM row (R*p + r), feature f.
    r_cst = ub[0:CROWS].rearrange("(p r) f -> p (r f)", p=128)
    r_u = [
        ub[CROWS + g * CW:CROWS + (g + 1) * CW].rearrange(
            "(p r) f -> p (r f)", p=128
        )
        for g in range(NCH)
    ]

    with tile.TileContext(nc) as tc:
        with (
            tc.tile_pool(name="const", bufs=1) as cpool,
            tc.tile_pool(name="big", bufs=1) as bpool,
            tc.tile_pool(name="w", bufs=2) as wpool,
            tc.tile_pool(name="d", bufs=2) as dpool,
            tc.tile_pool(name="yst", bufs=1) as ypool,
            tc.tile_pool(name="ps", bufs=1, space="PSUM") as pspool,
            tc.tile_pool(name="tst", bufs=2, space="PSUM") as tpool,
        ):
            # -- on-device identity (for PE block transposes) + tanh
            #    table warm-up, all during the DMA-in window --
            ones = cpool.tile([128, 128], F16, tag="ones")
            nc.gpsimd.memset(ones[:], 1.0)
            ident = cpool.tile([128, 128], F16, tag="ident")
            nc.gpsimd.affine_select(
                ident[:], ones[:], pattern=[[-1, 128]], compare_op=EQ,
                fill=0.0, base=0, channel_multiplier=1,
            )
            tiny = cpool.tile([128, 1], F32, tag="tiny")
            nc.gpsimd.memset(tiny[:], 0.0)
            tiny2 = cpool.tile([128, 1], F32, tag="tiny2")
            nc.scalar.activation(tiny2[:], tiny[:], TANH)

            # -- input DMAs: u chunks on the SP ring (so chunk 0 lands
            #    first), constants concurrently on the ACT ring --
            ustage = bpool.tile([128, BC], F16, tag="ustage")
            for g in range(NCH):
                nc.sync.dma_start(ustage[:, g * CW:(g + 1) * CW], r_u[g])
            cstage = cpool.tile([128, CROWS], F16, tag="cstage")
            nc.scalar.dma_start(cstage[:], r_cst)

            cblk = lambda j: cstage[:, j * 128:(j + 1) * 128]
            d12lt = cpool.tile([128, 128], F16, tag="d12lt")
            ltr = cpool.tile([128, 128], F16, tag="ltr")
            gut = cpool.tile([128, 128], F16, tag="gut")
            gwt = cpool.tile([128, 128], F16, tag="gwt")
            btile = cpool.tile([128, 4], F16, tag="btile")
            xcl = btile[:, 0:2].bitcast(F32)  # xc/Lam       [128,1] f32
            c0 = btile[:, 2:4].bitcast(F32)   # C2 Einv F x0 [128,1] f32

            ps = [
                pspool.tile([128, CW], F32, tag=f"ps{n}", name=f"ps{n}")
                for n in range(NCH)
            ]
            ut = bpool.tile([128, BC], F16, tag="ut")

            w_cur = [None] * NCH
            w_prev = [None] * NCH

            def load_chunk(n):
                # 4 block transposes + one packed copy -> ut chunk n
                tpu = tpool.tile([128, CW], F16, tag="tst", name=f"tpu{n}")
                for k in range(4):
                    nc.tensor.transpose(
                        tpu[:, k * 128:(k + 1) * 128],
                        ustage[:, (4 * n + k) * 128:(4 * n + k + 1) * 128],
                        ident[:],
                    )
                sl = slice(n * CW, (n + 1) * CW)
                nc.vector.tensor_copy(ut[:, sl], tpu[:])

            def seed(n):
                sl = slice(n * CW, (n + 1) * CW)
                nc.tensor.matmul(ps[n][:], d12lt[:], ut[:, sl],
                                 start=True, stop=False)
                wt = wpool.tile([128, CW], F16, tag=f"w{n}", name=f"w{n}")
                nc.scalar.activation(wt[:], ps[n][:], TANH, bias=xcl)
                w_cur[n] = wt

            # chunk 0 lands first; seed weights + bias right behind it;
            # ltr before the first pass needs it; gut/gwt deferred.
            load_chunk(0)
            tpa = tpool.tile([128, CW], F16, tag="tst", name="tpa")
            nc.tensor.transpose(tpa[:, 0:128], cblk(0), ident[:])
            nc.tensor.transpose(tpa[:, 128:256], cblk(4), ident[:])
            nc.vector.tensor_copy(d12lt[:], tpa[:, 0:128])
            nc.vector.tensor_copy(btile[:], tpa[:, 128:132])
            seed(0)
            load_chunk(1)
            tpb = tpool.tile([128, CW], F16, tag="tst", name="tpb")
            nc.tensor.transpose(tpb[:, 0:128], cblk(1), ident[:])
            nc.vector.tensor_copy(ltr[:], tpb[:, 0:128])
            seed(1)
            load_chunk(2)
            seed(2)
            load_chunk(3)
            seed(3)
            nc.tensor.transpose(tpb[:, 128:256], cblk(2), ident[:])
            nc.tensor.transpose(tpb[:, 256:384], cblk(3), ident[:])
            nc.vector.tensor_copy(gut[:], tpb[:, 128:256])
            nc.vector.tensor_copy(gwt[:], tpb[:, 256:384])
            # chunks 2/3 get dedicated po banks so Gu@Ut runs early and
            # their tails shrink to gwt+add+store.
            po23 = [
                tpool.tile([128, CW], F32, tag=f"po{n}", name=f"po{n}",
                           bufs=1)
                for n in (2, 3)
            ]
            for n in (2, 3):
                sl = slice(n * CW, (n + 1) * CW)
                nc.tensor.matmul(po23[n - 2][:], gut[:], ut[:, sl],
                                 start=True, stop=False)

            # -- delta-Jacobi passes: ps += Lhat @ (W_k - W_{k-1}) --
            for m in range(M_PASSES):
                last = m == M_PASSES - 1
                for n in range(NCH):
                    if m == 0:
                        dl = w_cur[n]  # W1 - 0
                    else:
                        dl = dpool.tile([128, CW], F16, tag=f"d{n}",
                                        name=f"d{n}")
                        nc.vector.tensor_sub(dl[:], w_cur[n][:], w_prev[n][:])
                    nc.tensor.matmul(ps[n][:], ltr[:], dl[:],
                                     start=False, stop=last)
                for n in range(NCH):
                    w_prev[n] = w_cur[n]
                    wt = wpool.tile([128, CW], F16, tag=f"w{n}", name=f"w{n}")
                    nc.scalar.activation(wt[:], ps[n][:], TANH, bias=xcl)
                    w_cur[n] = wt

            # -- per-chunk tail: po = Gu@Ut + Gw@W; yt = po + c0; store.
            #    Chunks 0/1 reuse their freed ps banks (their tanhs end
            #    earliest); c0-adds alternate DVE / ACT-Identity (same
            #    act table as tanh, no reload) --
            for n in range(NCH):
                sl = slice(n * CW, (n + 1) * CW)
                if n < 2:
                    po = pspool.tile([128, CW], F32, tag=f"ps{n}",
                                     name=f"po{n}")
                    nc.tensor.matmul(po[:], gut[:], ut[:, sl],
                                     start=True, stop=False)
                else:
                    po = po23[n - 2]
                nc.tensor.matmul(po[:], gwt[:], w_cur[n][:],
                                 start=False, stop=True)
                yts = ypool.tile([128, CW], F16, tag=f"yt{n}", name=f"yt{n}")
                if n % 2 == 0:
                    nc.vector.tensor_scalar_add(yts[:], po[:], c0)
                else:
                    nc.scalar.activation(yts[:], po[:], IDENT, bias=c0)
                nc.sync.dma_start(y[:, sl], yts[:])
    nc.compile()
    return nc


def _derive_consts(X, Y, B2, C2, D21, D22, D12, x0):
    """Fold the contractive parameterization into kernel constants.
    Returns the [CROWS, 128] f16 constant region (matrix j's row m at
    DRAM row 5m + j; bias rows at 5p + 4)."""
    f = np.float32
    X = np.ascontiguousarray(X, f)
    H = (X.T @ X + EPS * np.eye(DIM_H, dtype=f)).astype(f)
    H11 = H[:DIM_X, :DIM_X]
    H21 = H[DIM_X:DIM_X + DIM_NL, :DIM_X]
    H22 = H[DIM_X:DIM_X + DIM_NL, DIM_X:DIM_X + DIM_NL]
    H31 = H[DIM_X + DIM_NL:, :DIM_X]
    H32 = H[DIM_X + DIM_NL:, DIM_X:DIM_X + DIM_NL]
    H33 = H[DIM_X + DIM_NL:, DIM_X + DIM_NL:]
    F = H31
    B1 = H32
    E = (0.5 * (H11 + ALPHA * H33 + Y - Y.T)).astype(f)
    Lam = (0.5 * np.diagonal(H22)).astype(f)
    D11 = (-np.tril(H22, k=-1)).astype(f)
    C1 = -H21

    Einv = np.linalg.inv(E).astype(f)
    x0v = np.asarray(x0, f)[0, 0, :]
    xc = (C1 @ x0v).astype(f)
    fx = (F @ x0v).astype(f)

    Lhat = (D11 / Lam[:, None]).astype(f)
    D12L = (np.asarray(D12, f) / Lam[:, None]).astype(f)
    CE = (np.asarray(C2, f) @ Einv).astype(f)
    Gu = (CE @ B2 + D22).astype(f)
    Gw = (CE @ B1 + D21).astype(f)
    xclam = (xc / Lam).astype(f)
    c0 = (CE @ fx).astype(f)

    h = np.float16
    V = np.zeros((128, 5, 128), h)
    V[:, 0] = D12L.astype(h)
    V[:, 1] = Lhat.astype(h)
    V[:, 2] = Gu.astype(h)
    V[:, 3] = Gw.astype(h)
    xb = xclam.view(np.uint32)
    cb = c0.view(np.uint32)
    V[0, 4] = (xb & 0xFFFF).astype(np.uint16).view(h)
    V[1, 4] = (xb >> 16).astype(np.uint16).view(h)
    V[2, 4] = (cb & 0xFFFF).astype(np.uint16).view(h)
    V[3, 4] = (cb >> 16).astype(np.uint16).view(h)
    return V.reshape(CROWS, 128)


def _make_in_maps(u_in, X, Y, B2, C2, D21, D22, D12, x0):
    cst = _derive_consts(X, Y, B2, C2, D21, D22, D12, x0)
    u16 = np.asarray(u_in, np.float32).reshape(B, DIM_IN).astype(np.float16)
    maps = []
    for i in range(N_CORES):
        uc = u16[i * BC:(i + 1) * BC]
        # per chunk: partition p holds rows {4p + r} = batch {128r + p}
        S = uc.reshape(NCH, 4, 128, DIM_IN).transpose(0, 2, 1, 3)
        ubuf = np.concatenate([cst, S.reshape(BC, DIM_IN)], axis=0)
        maps.append({"ub": np.ascontiguousarray(ubuf)})
    return maps


def kernel(u_in, X, Y, B2, C2, D21, D22, D12, x0):
    if "nc" not in _BUILT:
        _BUILT["nc"] = _build_nc()
    nc = _BUILT["nc"]
    in_maps = _make_in_maps(u_in, X, Y, B2, C2, D21, D22, D12, x0)
    res = run_bass_kernel_spmd(nc, in_maps, core_ids=list(range(N_CORES)))
    out = np.concatenate(
        [res.results[i]["y"].astype(np.float32).T for i in range(N_CORES)],
        axis=0,
    )
    return out.reshape(B, 1, DIM_OUT)


# revision 17
# speedup vs baseline: 1.0495x; 1.0495x over previous
"""Trainium2 Bass kernel for the ContractiveREN problem.

Strategy
--------
Data parallel over the batch: each of the 8 NeuronCores gets a 2048-row
shard of ``u_in``; all (small) parameter matrices are folded on the host
into four 128x128 fp16 matmul weights plus two per-partition fp32 bias
vectors.

Math
----
The reference computes (per batch row u, with x0 the initial state):
    w_i   = tanh((xc_i + ud_i + sum_{j<i} D11_ij w_j) / Lam_i)   (i = 0..127)
    y     = u @ Gu^T + w @ Gw^T + c0
where everything except the w-recurrence is affine in (u, w) and folds into
    Lhat = D11 / Lam[:,None],  xcl = xc/Lam,  UD = (D12/Lam) @ u^T
    Gu   = C2 @ inv(E) @ B2 + D22,  Gw = C2 @ inv(E) @ B1 + D21
    c0   = C2 @ inv(E) @ F @ x0
The strictly-lower-triangular recurrence is solved by fixed-point
iteration  W <- tanh(Lhat @ W + UD + xcl), which contracts the error by
~3.2x per sweep; 3 sweeps give rel err ~3.4e-3 against the fp32
reference (the harness gate is 2e-2; measured hw error matches the host
fp16 simulation exactly).

Implementation notes (all fp16 data / fp32 PSUM accumulation):
  * Input loads avoid both row-granular DMA (descriptor-rate bound) and
    the XBAR DMA transpose (slow, and serializing against every other
    DMA): the host pre-scatters u and the constants so a handful of
    big-descriptor DMAs land them in SBUF as 128x128 blocks, which PE
    block-transposes (identity generated on device via affine_select)
    and cheap contiguous DVE copies reassemble feature-major. Each
    512-batch chunk rides its own DMA so the pipeline starts on the
    first chunk while the rest are in flight.
  * Each 512-batch chunk owns one live PSUM bank holding
    UD + Lhat@W_k: pass k accumulates Lhat@(W_k - W_{k-1}) via matmul
    (the W-delta is a cheap all-fp16 DVE subtract), and every tanh
    applies xcl as the ACT bias, so no UDb tensor and no fp32 adds.
  * Output is computed feature-major at each chunk's tail (Gu@Ut +
    Gw@W accumulated into the chunk's freed ps bank, + c0 via DVE
    tensor-scalar / ACT-Identity-bias alternating) and stored to DRAM
    feature-major as fp16; the host transposes and upcasts per core.
"""

import numpy as np

import concourse.bass as bass
import concourse.mybir as mybir
import concourse.tile as tile
from concourse import bacc
from concourse.bass_utils import run_bass_kernel_spmd

B = 16384
N_CORES = 8
BC = B // N_CORES  # 2048 batch rows per core
DIM_IN = 128
DIM_OUT = 128
DIM_X = 512
DIM_NL = 128
DIM_H = 2 * DIM_X + DIM_NL
EPS = 1e-3
ALPHA = 1.0
NCH = 4            # 512-column batch chunks (one PSUM bank each)
CW = BC // NCH     # 512
M_PASSES = 2       # delta-Jacobi passes after the seeded first sweep
CROWS = 5 * 128    # constant region: D12L | Lhat | Gu | Gw | bias blocks
F16 = mybir.dt.float16
F32 = mybir.dt.float32
TANH = mybir.ActivationFunctionType.Tanh
IDENT = mybir.ActivationFunctionType.Identity
EQ = mybir.AluOpType.is_equal

_BUILT = {}


def _build_nc():
    nc = bacc.Bacc("TRN2", target_bir_lowering=False, debug=False)
    ub = nc.dram_tensor(
        "ub", [CROWS + BC, DIM_IN], F16, kind="ExternalInput"
    ).ap()
    y = nc.dram_tensor("y", [DIM_OUT, BC], F16, kind="ExternalOutput").ap()

    # big-descriptor views: partition p reads R consecutive DRAM rows,
    # giving SBUF col (r*128 + f) = DRAM row (R*p + r), feature f.
    r_cst = ub[0:CROWS].rearrange("(p r) f -> p (r f)", p=128)
    r_u = [
        ub[CROWS + g * CW:CROWS + (g + 1) * CW].rearrange(
            "(p r) f -> p (r f)", p=128
        )
        for g in range(NCH)
    ]

    with tile.TileContext(nc) as tc:
        with (
            tc.tile_pool(name="const", bufs=1) as cpool,
            tc.tile_pool(name="big", bufs=1) as bpool,
            tc.tile_pool(name="w", bufs=2) as wpool,
            tc.tile_pool(name="d", bufs=2) as dpool,
            tc.tile_pool(name="yst", bufs=1) as ypool,
            tc.tile_pool(name="ps", bufs=1, space="PSUM") as pspool,
            tc.tile_pool(name="tst", bufs=2, space="PSUM") as tpool,
        ):
            # -- on-device identity (for PE block transposes) + tanh
            #    table warm-up, all during the DMA-in window --
            ones = cpool.tile([128, 128], F16, tag="ones")
            nc.gpsimd.memset(ones[:], 1.0)
            ident = cpool.tile([128, 128], F16, tag="ident")
            nc.gpsimd.affine_select(
                ident[:], ones[:], pattern=[[-1, 128]], compare_op=EQ,
                fill=0.0, base=0, channel_multiplier=1,
            )
            tiny = cpool.tile([128, 1], F32, tag="tiny")
            nc.gpsimd.memset(tiny[:], 0.0)
            tiny2 = cpool.tile([128, 1], F32, tag="tiny2")
            nc.scalar.activation(tiny2[:], tiny[:], TANH)

            # -- input DMAs, all on the SP ring in FIFO order (splitting
            #    across rings just contends for the same 16 DMA engines) --
            cstage = cpool.tile([128, CROWS], F16, tag="cstage")
            nc.sync.dma_start(cstage[:], r_cst)
            ustage = bpool.tile([128, BC], F16, tag="ustage")
            for g in range(NCH):
                nc.sync.dma_start(ustage[:, g * CW:(g + 1) * CW], r_u[g])

            cblk = lambda j: cstage[:, j * 128:(j + 1) * 128]
            d12lt = cpool.tile([128, 128], F16, tag="d12lt")
            ltr = cpool.tile([128, 128], F16, tag="ltr")
            gut = cpool.tile([128, 128], F16, tag="gut")
            gwt = cpool.tile([128, 128], F16, tag="gwt")
            btile = cpool.tile([128, 4], F16, tag="btile")
            xcl = btile[:, 0:2].bitcast(F32)  # xc/Lam       [128,1] f32
            c0 = btile[:, 2:4].bitcast(F32)   # C2 Einv F x0 [128,1] f32

            ps = [
                pspool.tile([128, CW], F32, tag=f"ps{n}", name=f"ps{n}")
                for n in range(NCH)
            ]
            ut = bpool.tile([128, BC], F16, tag="ut")

            w_cur = [None] * NCH
            w_prev = [None] * NCH

            def load_chunk(n):
                # 4 block transposes + one packed copy -> ut chunk n
                tpu = tpool.tile([128, CW], F16, tag="tst", name=f"tpu{n}")
                for k in range(4):
                    nc.tensor.transpose(
                        tpu[:, k * 128:(k + 1) * 128],
                        ustage[:, (4 * n + k) * 128:(4 * n + k + 1) * 128],
                        ident[:],
                    )
                sl = slice(n * CW, (n + 1) * CW)
                nc.vector.tensor_copy(ut[:, sl], tpu[:])

            def seed(n):
                sl = slice(n * CW, (n + 1) * CW)
                nc.tensor.matmul(ps[n][:], d12lt[:], ut[:, sl],
                                 start=True, stop=False)
                wt = wpool.tile([128, CW], F16, tag=f"w{n}", name=f"w{n}")
                nc.scalar.activation(wt[:], ps[n][:], TANH, bias=xcl)
                w_cur[n] = wt

            # constants land first; seed weights + bias, then chunk 0;
            # ltr before the first pass needs it; gut/gwt deferred.
            tpa = tpool.tile([128, CW], F16, tag="tst", name="tpa")
            nc.tensor.transpose(tpa[:, 0:128], cblk(0), ident[:])
            nc.tensor.transpose(tpa[:, 128:256], cblk(4), ident[:])
            nc.vector.tensor_copy(d12lt[:], tpa[:, 0:128])
            nc.vector.tensor_copy(btile[:], tpa[:, 128:132])
            load_chunk(0)
            seed(0)
            load_chunk(1)
            tpb = tpool.tile([128, CW], F16, tag="tst", name="tpb")
            nc.tensor.transpose(tpb[:, 0:128], cblk(1), ident[:])
            nc.vector.tensor_copy(ltr[:], tpb[:, 0:128])
            seed(1)
            load_chunk(2)
            seed(2)
            load_chunk(3)
            seed(3)
            nc.tensor.transpose(tpb[:, 128:256], cblk(2), ident[:])
            nc.tensor.transpose(tpb[:, 256:384], cblk(3), ident[:])
            nc.vector.tensor_copy(gut[:], tpb[:, 128:256])
            nc.vector.tensor_copy(gwt[:], tpb[:, 256:384])
            # chunks 2/3 get dedicated po banks so Gu@Ut runs early and
            # their tails shrink to gwt+add+store.
            po23 = [
                tpool.tile([128, CW], F32, tag=f"po{n}", name=f"po{n}",
                           bufs=1)
                for n in (2, 3)
            ]
            for n in (2, 3):
                sl = slice(n * CW, (n + 1) * CW)
                nc.tensor.matmul(po23[n - 2][:], gut[:], ut[:, sl],
                                 start=True, stop=False)

            # -- delta-Jacobi passes: ps += Lhat @ (W_k - W_{k-1}) --
            for m in range(M_PASSES):
                last = m == M_PASSES - 1
                for n in range(NCH):
                    if m == 0:
                        dl = w_cur[n]  # W1 - 0
                    else:
                        dl = dpool.tile([128, CW], F16, tag=f"d{n}",
                                        name=f"d{n}")
                        nc.vector.tensor_sub(dl[:], w_cur[n][:], w_prev[n][:])
                    nc.tensor.matmul(ps[n][:], ltr[:], dl[:],
                                     start=False, stop=last)
                for n in range(NCH):
                    w_prev[n] = w_cur[n]
                    wt = wpool.tile([128, CW], F16, tag=f"w{n}", name=f"w{n}")
                    nc.scalar.activation(wt[:], ps[n][:], TANH, bias=xcl)
                    w_cur[n] = wt

            # -- per-chunk tail: po = Gu@Ut + Gw@W; yt = po + c0; store.
            #    Chunks 0/1 reuse their freed ps banks (their tanhs end
            #    earliest); c0-adds alternate DVE / ACT-Identity (same
            #    act table as tanh, no reload) --
            for n in range(NCH):
                sl = slice(n * CW, (n + 1) * CW)
                if n < 2:
                    po = pspool.tile([128, CW], F32, tag=f"ps{n}",
                                     name=f"po{n}")
                    nc.tensor.matmul(po[:], gut[:], ut[:, sl],
                                     start=True, stop=False)
                else:
                    po = po23[n - 2]
                nc.tensor.matmul(po[:], gwt[:], w_cur[n][:],
                                 start=False, stop=True)
                yts = ypool.tile([128, CW], F16, tag=f"yt{n}", name=f"yt{n}")
                if n % 2 == 0:
                    nc.vector.tensor_scalar_add(yts[:], po[:], c0)
                else:
                    nc.scalar.activation(yts[:], po[:], IDENT, bias=c0)
                nc.sync.dma_start(y[:, sl], yts[:])
    nc.compile()
    return nc


def _derive_consts(X, Y, B2, C2, D21, D22, D12, x0):
    """Fold the contractive parameterization into kernel constants.
    Returns the [CROWS, 128] f16 constant region (matrix j's row m at
    DRAM row 5m + j; bias rows at 5p + 4)."""
    f = np.float32
    X = np.ascontiguousarray(X, f)
    H = (X.T @ X + EPS * np.eye(DIM_H, dtype=f)).astype(f)
    H11 = H[:DIM_X, :DIM_X]
    H21 = H[DIM_X:DIM_X + DIM_NL, :DIM_X]
    H22 = H[DIM_X:DIM_X + DIM_NL, DIM_X:DIM_X + DIM_NL]
    H31 = H[DIM_X + DIM_NL:, :DIM_X]
    H32 = H[DIM_X + DIM_NL:, DIM_X:DIM_X + DIM_NL]
    H33 = H[DIM_X + DIM_NL:, DIM_X + DIM_NL:]
    F = H31
    B1 = H32
    E = (0.5 * (H11 + ALPHA * H33 + Y - Y.T)).astype(f)
    Lam = (0.5 * np.diagonal(H22)).astype(f)
    D11 = (-np.tril(H22, k=-1)).astype(f)
    C1 = -H21

    Einv = np.linalg.inv(E).astype(f)
    x0v = np.asarray(x0, f)[0, 0, :]
    xc = (C1 @ x0v).astype(f)
    fx = (F @ x0v).astype(f)

    Lhat = (D11 / Lam[:, None]).astype(f)
    D12L = (np.asarray(D12, f) / Lam[:, None]).astype(f)
    CE = (np.asarray(C2, f) @ Einv).astype(f)
    Gu = (CE @ B2 + D22).astype(f)
    Gw = (CE @ B1 + D21).astype(f)
    xclam = (xc / Lam).astype(f)
    c0 = (CE @ fx).astype(f)

    h = np.float16
    V = np.zeros((128, 5, 128), h)
    V[:, 0] = D12L.astype(h)
    V[:, 1] = Lhat.astype(h)
    V[:, 2] = Gu.astype(h)
    V[:, 3] = Gw.astype(h)
    xb = xclam.view(np.uint32)
    cb = c0.view(np.uint32)
    V[0, 4] = (xb & 0xFFFF).astype(np.uint16).view(h)
    V[1, 4] = (xb >> 16).astype(np.uint16).view(h)
    V[2, 4] = (cb & 0xFFFF).astype(np.uint16).view(h)
    V[3, 4] = (cb >> 16).astype(np.uint16).view(h)
    return V.reshape(CROWS, 128)


def _make_in_maps(u_in, X, Y, B2, C2, D21, D22, D12, x0):
    cst = _derive_consts(X, Y, B2, C2, D21, D22, D12, x0)
    u16 = np.asarray(u_in, np.float32).reshape(B, DIM_IN).astype(np.float16)
    maps = []
    for i in range(N_CORES):
        uc = u16[i * BC:(i + 1) * BC]
        # per chunk: partition p holds rows {4p + r} = batch {128r + p}
        S = uc.reshape(NCH, 4, 128, DIM_IN).transpose(0, 2, 1, 3)
        ubuf = np.concatenate([cst, S.reshape(BC, DIM_IN)], axis=0)
        maps.append({"ub": np.ascontiguousarray(ubuf)})
    return maps


def kernel(u_in, X, Y, B2, C2, D21, D22, D12, x0):
    if "nc" not in _BUILT:
        _BUILT["nc"] = _build_nc()
    nc = _BUILT["nc"]
    in_maps = _make_in_maps(u_in, X, Y, B2, C2, D21, D22, D12, x0)
    res = run_bass_kernel_spmd(nc, in_maps, core_ids=list(range(N_CORES)))
    out = np.concatenate(
        [res.results[i]["y"].astype(np.float32).T for i in range(N_CORES)],
        axis=0,
    )
    return out.reshape(B, 1, DIM_OUT)
